# revision 26
# baseline (speedup 1.0000x reference)
"""Multi-head self-attention (RMSNorm + causal MHA + out-proj) on 8 TRN2 cores.

Sharding (per the tensor-parallel hint): core c handles batch b = c//4 and
head group hg = c%4 (4 of 16 heads). Each core computes a PARTIAL output (its
heads' slice of the out-projection contraction); the host sums the 4 partials
per batch and transposes back.

Device kernel (per core, feature-major / transposed orientation throughout so
no on-chip transposes are ever needed):
  - RMSNorm via an all-ones 128x128 matmul (partition reduce + broadcast to
    all partitions in one shot); rstd via ACT sqrt + DVE reciprocal.
  - bf16 compute on TensorE. Norm weight folded into the projection weights
    on the host (exact); weights shipped pre-transposed bf16 in flat
    SBUF-layout so every input is a single contiguous DMA.
  - Q^T/K^T stored once with a head PAIR stacked on partition halves; score
    matmuls for the odd head read at base partition 64 (no duplication).
  - Causal handling: score matmuls and exp cover only the causal region;
    diagonal-tile regions are packed contiguously in PSUM so exp runs in two
    large ACT ops per chunk. In-block masks via affine_select on the
    otherwise-idle Pool engine.
  - Softmax denominator fused into PV via a ones column in V (M=65); ctx is
    normalized directly out of PSUM: DVE reciprocal of the l row, Pool
    partition_broadcast of 1/l, DVE multiply.
  - The attention stream is ACT(exp)-bound, so independent PE work (pair-1
    QK projection during head 1, out-projection chunks during head 3) is
    emitted as small filler units BETWEEN attention groups: the PE sequencer
    is in-order, so fillers sit exactly where the score matmul would
    otherwise block on the exp pipeline.
"""

import os
from contextlib import ExitStack

import numpy as np
import ml_dtypes

import concourse.bass as bass
import concourse.tile as tile
from concourse import bacc, mybir
from concourse.bass_utils import run_bass_kernel_spmd

F32 = mybir.dt.float32
BF16 = mybir.dt.bfloat16
AF = mybir.ActivationFunctionType
P = 128
DD = 64
T = 2048
D = 1024
NH = 4            # heads per core
KT = D // P       # 8 feature tiles
TT = T // P       # 16 token tiles
TC = T // 512     # 4 query chunks
N_CORES = 8
EPS = 1e-6


def build_kernel(nc, reps=1):
    xT_d = nc.dram_tensor("xT", [P, KT * T], BF16, kind="ExternalInput")
    wqk_d = nc.dram_tensor("wqkT", [P, KT * 512], BF16, kind="ExternalInput")
    wv_d = nc.dram_tensor("wvT", [P, KT * 256], BF16, kind="ExternalInput")
    wo_d = nc.dram_tensor("woT", [P, 2 * D], BF16, kind="ExternalInput")
    outT_d = nc.dram_tensor("outT", [P, 8 * T], BF16, kind="ExternalOutput")

    with tile.TileContext(nc) as tc, ExitStack() as ctx:
        consts = ctx.enter_context(tc.tile_pool(name="consts", bufs=1))
        persist = ctx.enter_context(tc.tile_pool(name="persist", bufs=1))
        xsqp = ctx.enter_context(tc.tile_pool(name="xsqp", bufs=2))
        epool = ctx.enter_context(tc.tile_pool(name="epool", bufs=3))
        rlp = ctx.enter_context(tc.tile_pool(name="rlp", bufs=2))
        osbp = ctx.enter_context(tc.tile_pool(name="osbp", bufs=8))
        sps = ctx.enter_context(tc.tile_pool(name="sps", bufs=2, space="PSUM"))
        ctxp = ctx.enter_context(tc.tile_pool(name="ctxp", bufs=2, space="PSUM"))
        mmp = ctx.enter_context(tc.tile_pool(name="mmp", bufs=2, space="PSUM"))

        # ---- loop-invariant prelude: consts, weights, ones blocks ------
        ones_bf = consts.tile([P, P], BF16)
        nc.vector.memset(ones_bf[:], 1.0)
        eps_sb = consts.tile([P, 1], F32)
        nc.vector.memset(eps_sb[:], EPS)
        maskC = consts.tile([P, P], BF16)
        nc.gpsimd.memset(maskC[:], -640.0)
        nc.gpsimd.affine_select(
            out=maskC[:], in_=maskC[:],
            compare_op=mybir.AluOpType.is_ge, fill=0.0, base=-1,
            pattern=[[1, P]], channel_multiplier=-1,
        )
        ident = consts.tile([P, P], BF16)
        nc.gpsimd.memset(ident[:], 1.0)
        nc.gpsimd.affine_select(
            out=ident[:], in_=ident[:],
            compare_op=mybir.AluOpType.is_ge, fill=0.0, base=0,
            pattern=[[1, P]], channel_multiplier=-1,
        )
        nc.gpsimd.affine_select(
            out=ident[:], in_=ident[:],
            compare_op=mybir.AluOpType.is_ge, fill=0.0, base=0,
            pattern=[[-1, P]], channel_multiplier=1,
        )

        xbf = persist.tile([P, KT, T], BF16)
        xn = persist.tile([P, KT, T], BF16)
        rstd_bf = persist.tile([P, T], BF16)
        wqk_bf = persist.tile([P, KT, 512], BF16)
        wv_bf = persist.tile([P, KT, 256], BF16)
        wo_bf = persist.tile([P, 2, D], BF16)
        QTd = persist.tile([P, 2, T], BF16)
        KTd = persist.tile([P, 2, T], BF16)
        Vsb = persist.tile([P, NH, TT, P], BF16)
        ctxn = persist.tile([P, 2, T], BF16)

        Vhp = Vsb[:, :, :, :].rearrange("p (a b) t d -> p a b t d", b=2)
        nc.gpsimd.memset(Vhp[:, :, 0, :, 64:128], 1.0)
        nc.gpsimd.memset(Vhp[:, :, 1, :, 0:64], 1.0)
        nc.sync.dma_start(wqk_bf[:], wqk_d.ap())
        nc.sync.dma_start(wv_bf[:], wv_d.ap())
        nc.sync.dma_start(wo_bf[:], wo_d.ap())

        def x_dma(kt):
            nc.sync.dma_start(xbf[:, kt, :], xT_d.ap()[:, kt * T : (kt + 1) * T])

        def emit_body(iv=None, skip_x=False, prefetch_next=False):
            # ---- load x + RMSNorm --------------------------------------
            ms_h = [
                sps.tile([P, 1024], F32, tag="big", name=f"ms{_i}") for _i in range(2)
            ]
            for kt in range(KT):
                if not skip_x:
                    x_dma(kt)
                xsq = xsqp.tile([P, T], BF16)
                nc.vector.tensor_mul(xsq[:], xbf[:, kt, :], xbf[:, kt, :])
                for c in range(TC):
                    nc.tensor.matmul(
                        ms_h[c // 2][:, 512 * (c % 2) : 512 * (c % 2) + 512],
                        ones_bf[:], xsq[:, 512 * c : 512 * (c + 1)],
                        start=(kt == 0), stop=(kt == KT - 1),
                    )
            def emit_rstd_chunk(c):
                cs = slice(512 * c, 512 * (c + 1))
                sq = xsqp.tile([P, 512], F32, tag="sq")
                nc.scalar.activation(
                    sq[:], ms_h[c // 2][:, 512 * (c % 2) : 512 * (c % 2) + 512],
                    AF.Sqrt, bias=eps_sb[:, 0:1], scale=1.0 / D,
                )
                with nc.allow_low_precision(reason="rstd feeds bf16 matmuls"):
                    nc.vector.reciprocal(rstd_bf[:, cs], sq[:])

            def emit_xn_chunk(c):
                cs = slice(512 * c, 512 * (c + 1))
                for kt in range(KT):
                    nc.vector.tensor_mul(xn[:, kt, cs], xbf[:, kt, cs], rstd_bf[:, cs])

            # ---- projections -------------------------------------------
            qk_state = {}

            def qk_half(pair, is_k, c, second):
                """Half of one q/k chunk projection (4 of 8 kt matmuls)."""
                ft = 2 * is_k + pair
                key = (pair, is_k, c)
                if not second:
                    qk_state[key] = mmp.tile([P, 512], F32, tag="mm", name="qkps")
                qkps = qk_state[key]
                for kt in range(4 * second, 4 * second + 4):
                    nc.tensor.matmul(
                        qkps[:],
                        wqk_bf[:, kt, P * ft : P * (ft + 1)],
                        xn[:, kt, 512 * c : 512 * (c + 1)],
                        start=(kt == 0), stop=(kt == KT - 1),
                    )
                if second:
                    dst = KTd if is_k else QTd
                    nc.vector.tensor_copy(
                        dst[:, pair, 512 * c : 512 * (c + 1)], qkps[:]
                    )
                    del qk_state[key]

            def emit_qk_chunk(pair, is_k, c):
                qk_half(pair, is_k, c, 0)
                qk_half(pair, is_k, c, 1)

            def emit_v_tiles(t0, t1):
                for tt in range(t0, t1):
                    vps = mmp.tile([P, 512], F32, tag="mm")
                    for kt in range(KT):
                        nc.tensor.matmul(
                            vps[:, :256],
                            xn[:, kt, P * tt : P * (tt + 1)],
                            wv_bf[:, kt, :],
                            start=(kt == 0), stop=(kt == KT - 1),
                        )
                    vh = vps[:, 0:256].rearrange("p (a b d) -> p a b d", a=2, d=64)
                    nc.vector.tensor_copy(Vhp[:, :, 0, tt, 0:64], vh[:, :, 0, :])
                    nc.vector.tensor_copy(Vhp[:, :, 1, tt, 64:128], vh[:, :, 1, :])

            # ---- out-projection units ----------------------------------
            osb_state = {}
            out_evac = [0]

            def outproj_unit(e, c):
                """One (e, c) column of the out-projection: 2 matmuls into a
                single-bank psum tile, evacuate into the staging half; on odd
                c DMA the staged [128,1024] c-pair out."""
                if c % 2 == 0:
                    osb_state[e] = osbp.tile([P, 1024], BF16, tag="osb", name="osb")
                osb = osb_state[e]
                ops = mmp.tile([P, 512], F32, tag="mm", name="ops")
                for ct in range(2):
                    nc.tensor.matmul(
                        ops[:],
                        wo_bf[:, ct, P * e : P * (e + 1)],
                        ctxn[:, ct, 512 * c : 512 * (c + 1)],
                        start=(ct == 0), stop=(ct == 1),
                    )
                half = 512 * (c % 2)
                if out_evac[0] % 2 == 0:
                    nc.vector.tensor_copy(osb[:, half : half + 512], ops[:])
                else:
                    nc.scalar.copy(osb[:, half : half + 512], ops[:])
                out_evac[0] += 1
                if c % 2 == 1:
                    nc.sync.dma_start(
                        outT_d.ap()[
                            :, T * e + 1024 * (c // 2) : T * e + 1024 * (c // 2) + 1024
                        ],
                        osb[:],
                    )
                    del osb_state[e]

            # ---- filler pump -------------------------------------------
            fillers = []

            def pump(n=1):
                for _ in range(n):
                    if not fillers:
                        return
                    fillers.pop(0)()

            def drain():
                while fillers:
                    fillers.pop(0)()

            # ---- attention ---------------------------------------------
            # PV matmuls and the norm chain are deferred through `post`: they
            # are emitted AFTER the next group's score matmuls, so the PE
            # never head-of-line blocks on the exp they consume.
            post = []

            def drain_post():
                while post:
                    post.pop(0)()

            def emit_attn_chunk(h, c):
                pair, half = h // 2, h % 2
                rg = DD * half
                Q = QTd[rg : rg + DD, pair, :]
                K_ = KTd[rg : rg + DD, pair, :]
                ctx_ps = ctxp.tile([P, 512], F32, tag="ctx")
                q0 = 512 * c

                def pv(e_t, j, coff, ccols, ecols, stop):
                    nc.tensor.matmul(
                        ctx_ps[:, coff : coff + ccols],
                        Vsb[:, h, j, 0:P],
                        e_t[:, ecols[0] : ecols[1]],
                        start=(j == 0), stop=stop,
                    )

                for g in range(2 * c):
                    j0, j1 = 2 * g, 2 * g + 1
                    sst = sps.tile([P, 1024], F32, tag="big")
                    nc.tensor.matmul(
                        sst[:, 0:512], K_[:, P * j0 : P * (j0 + 1)],
                        Q[:, q0 : q0 + 512], start=True, stop=True,
                    )
                    nc.tensor.matmul(
                        sst[:, 512:1024], K_[:, P * j1 : P * (j1 + 1)],
                        Q[:, q0 : q0 + 512], start=True, stop=True,
                    )
                    drain_post()
                    expS = epool.tile([P, 1024], BF16)
                    nc.scalar.activation(expS[:], sst[:], AF.Exp, scale=0.125)
                    post.append(lambda e_t=expS, a=j0, b=j1: (
                        pv(e_t, a, 0, 512, (0, 512), False),
                        pv(e_t, b, 0, 512, (512, 1024), False),
                    ))
                    pump()

                # diagonal group A: tiles 4c (W=512 @ col 0), 4c+1 (W=384 @ col 512)
                j0, j1 = 4 * c, 4 * c + 1
                sst = sps.tile([P, 1024], F32, tag="big")
                nc.tensor.matmul(
                    sst[:, 0:512], K_[:, P * j0 : P * (j0 + 1)],
                    Q[:, q0 : q0 + 512], start=True, stop=True,
                )
                nc.tensor.matmul(
                    sst[:, 512:896], K_[:, P * j1 : P * (j1 + 1)],
                    Q[:, q0 + 128 : q0 + 512], start=True, stop=True,
                )
                drain_post()
                expS = epool.tile([P, 1024], BF16)
                nc.scalar.activation(expS[:, 0:896], sst[:, 0:896], AF.Exp, scale=0.125)
                mA = expS[:, 0:1024].rearrange("p (a b d) -> p a b d", a=2, d=P)[
                    :, :, 0, :
                ]
                nc.gpsimd.affine_select(
                    out=mA, in_=mA,
                    compare_op=mybir.AluOpType.is_ge, fill=0.0, base=0,
                    pattern=[[0, 2], [1, P]], channel_multiplier=-1,
                )
                post.append(lambda e_t=expS, a=j0, b=j1: (
                    pv(e_t, a, 0, 512, (0, 512), False),
                    pv(e_t, b, 128, 384, (512, 896), False),
                ))
                pump()

                # diagonal group B: tiles 4c+2 (W=256 @ col 0), 4c+3 (W=128 @ col 256)
                j2, j3 = 4 * c + 2, 4 * c + 3
                sst = sps.tile([P, 1024], F32, tag="big")
                nc.tensor.matmul(
                    sst[:, 0:256], K_[:, P * j2 : P * (j2 + 1)],
                    Q[:, q0 + 256 : q0 + 512], start=True, stop=True,
                )
                nc.tensor.matmul(
                    sst[:, 256:384], K_[:, P * j3 : P * (j3 + 1)],
                    Q[:, q0 + 384 : q0 + 512], start=True, stop=True,
                )
                drain_post()
                expS = epool.tile([P, 1024], BF16)
                nc.scalar.activation(expS[:, 0:384], sst[:, 0:384], AF.Exp, scale=0.125)
                mB = expS[:, 0:512].rearrange("p (a b d) -> p a b d", a=2, d=P)[
                    :, :, 0, :
                ]
                nc.gpsimd.affine_select(
                    out=mB, in_=mB,
                    compare_op=mybir.AluOpType.is_ge, fill=0.0, base=0,
                    pattern=[[0, 2], [1, P]], channel_multiplier=-1,
                )
                post.append(lambda e_t=expS, a=j2, b=j3: (
                    pv(e_t, a, 256, 256, (0, 256), False),
                    pv(e_t, b, 384, 128, (256, 384), True),
                ))
                post.append(lambda: emit_norm(h, c, ctx_ps))
                pump()

            def emit_norm(h, c, ctx_ps):
                # even h: ctx rows [0:64], l replicated on [64:128]; odd h the
                # mirror. reciprocal the l rows, DMA-shift 1/l onto the ctx
                # rows' partitions, multiply straight into ctxn.
                pair, half = h // 2, h % 2
                cs = slice(512 * c, 512 * (c + 1))
                lrows = slice(DD, P) if half == 0 else slice(0, DD)
                crows = slice(0, DD) if half == 0 else slice(DD, P)
                rl = rlp.tile([P, 512], F32, tag="rl")
                nc.vector.reciprocal(rl[lrows, :], ctx_ps[lrows, :])
                nc.sync.dma_start(rl[crows, :], rl[lrows, :])
                with nc.allow_low_precision(reason="ctx feeds bf16 matmul"):
                    nc.vector.tensor_mul(
                        ctxn[crows, pair, cs], ctx_ps[crows, :], rl[crows, :]
                    )

            # ---- schedule: chunk-major ---------------------------------
            # Per chunk: normalize the token chunk, project q/k (both pairs)
            # and v, then attention for all 4 heads. Out-projection of chunk
            # c-1 pumps into chunk c's exp-bound attention gaps; chunk 3's
            # out-projection drains at the end.
            phase = os.environ.get("KERNEL_PHASE", "full")
            for c in range(TC):
                emit_rstd_chunk(c)
                emit_xn_chunk(c)
                if phase != "ldnorm":
                    for pair in range(2):
                        for is_k in (0, 1):
                            emit_qk_chunk(pair, is_k, c)
                    emit_v_tiles(4 * c, 4 * (c + 1))
                if phase in ("attn", "full"):
                    for h in (1, 3, 0, 2):
                        emit_attn_chunk(h, c)
                if phase == "full":
                    for e in range(D // P):
                        fillers.append(lambda ee=e, cc=c: outproj_unit(ee, cc))
                    if c == 2 and prefetch_next:
                        for kt in range(KT):
                            fillers.append(lambda k=kt: x_dma(k))
            if phase in ("attn", "full"):
                drain_post()
            drain()
            if phase != "full":
                dummy = osbp.tile([P, 1024], BF16, tag="osb", name="dummy")
                nc.vector.tensor_copy(dummy[:], xn[:, 0, 0:1024])
                nc.sync.dma_start(outT_d.ap()[:, 0:1024], dummy[:])

        if reps == 1:
            emit_body()
        else:
            unroll = 4 if reps % 4 == 0 else (2 if reps % 2 == 0 else 1)
            with tc.For_i(0, reps // unroll, 1) as iv:
                for _u in range(unroll):
                    emit_body(
                        iv, skip_x=(_u > 0), prefetch_next=(_u < unroll - 1)
                    )


_NC_CACHE = None


def _get_nc():
    global _NC_CACHE
    if _NC_CACHE is None:
        nc = bacc.Bacc(
            "TRN2", target_bir_lowering=False, debug=False, num_devices=N_CORES
        )
        build_kernel(nc)
        nc.compile()
        _NC_CACHE = nc
    return _NC_CACHE


def _tile_rows(a):
    """[R*128, C] -> [128, R*C] flat feature-tiled layout."""
    r, c = a.shape
    return np.ascontiguousarray(
        a.reshape(r // P, P, c).transpose(1, 0, 2).reshape(P, (r // P) * c)
    )


def make_in_maps(x, norm_weight, qkv_w, out_w):
    x = np.asarray(x, dtype=np.float32)
    norm_weight = np.asarray(norm_weight, dtype=np.float32)
    qkv_w = np.asarray(qkv_w, dtype=np.float32)
    out_w = np.asarray(out_w, dtype=np.float32)
    qkv_eff = qkv_w * norm_weight[None, :]
    bf = ml_dtypes.bfloat16
    in_maps = []
    for core in range(N_CORES):
        b, hg = core // 4, core % 4
        r0 = 256 * hg
        xT = _tile_rows(np.ascontiguousarray(x[b].T)).astype(bf)
        # wqk col blocks: [q-pair0, q-pair1, k-pair0, k-pair1] (128 each)
        wq = qkv_eff[r0 : r0 + 256]
        wk = qkv_eff[D + r0 : D + r0 + 256]
        wqk = np.concatenate([wq[:128], wq[128:], wk[:128], wk[128:]], 0).T
        wqkT = _tile_rows(np.ascontiguousarray(wqk)).astype(bf)
        wvT = _tile_rows(
            np.ascontiguousarray(qkv_eff[2 * D + r0 : 2 * D + r0 + 256].T)
        ).astype(bf)
        woT = _tile_rows(np.ascontiguousarray(out_w[:, r0 : r0 + 256].T)).astype(bf)
        in_maps.append({"xT": xT, "wqkT": wqkT, "wvT": wvT, "woT": woT})
    return in_maps


def gather_output(results):
    out = np.empty((2, T, D), np.float32)
    for b in range(2):
        acc = results[4 * b]["outT"].astype(np.float32).copy()
        for hg in range(1, 4):
            acc += results[4 * b + hg]["outT"]
        # [128, 8*T] -> [D, T] -> [T, D]
        out[b] = acc.reshape(P, D // P, T).transpose(1, 0, 2).reshape(D, T).T
    return out


def kernel(x, norm_weight, qkv_w, out_w):
    nc = _get_nc()
    in_maps = make_in_maps(x, norm_weight, qkv_w, out_w)
    res = run_bass_kernel_spmd(nc, in_maps, core_ids=list(range(N_CORES)))
    return gather_output(res.results)
